# revision 4
# baseline (speedup 1.0000x reference)
# Dissipation network Bass kernel for TRN2.
#
# Layout: each super-tile (ST) covers 2*F batch rows as 2 partition groups
# packed DENSELY: H-dim tensors at partitions 0:50 (PG0) / 50:100 (PG1),
# D-dim tensors at 0:16 / 16:32. Weights are block-diagonal bf16 tiles
# (diag(W, W)), so each weight needs ONE matmul instruction per 512-col
# chunk instead of one per PG (PE cost is output-free-size only).
# Activations (x path, xs path, s gates, x0/x0s) are bf16; PSUM fp32.
# Softplus = Exp (bias folded in) then Ln(t + 1), pinned to the
# natural_log_exp_and_others ACT table (single table load).
# Output-layer matmuls of 8 consecutive STs accumulate into one [16, F]
# PSUM tile so the final softplus runs once per 8 STs.
import numpy as np
import ml_dtypes
import concourse.bass as bass
from concourse import bacc
import concourse.hw_specs as hw_specs
import concourse.bacc as bacc_mod
import concourse.mybir as mybir
import concourse.tile as tile

dt = mybir.dt
AF = mybir.ActivationFunctionType
ALU = mybir.AluOpType

_orig_get_tables = hw_specs.get_activation_tables


def _pinned_tables(arch):
    t = _orig_get_tables(arch)
    out = {}
    for name, fns in t.items():
        if name != "natural_log_exp_and_others":
            fns = fns - {AF.Exp, AF.Ln}
        out[name] = fns
    return out


bacc_mod.get_activation_tables = _pinned_tables

D, H = 16, 50
F = 1024            # free columns per PG
STB = 2 * F         # batch rows per super-tile
NCHUNK = F // 512   # 512-col matmul chunks per F
NA = F // 128       # 128-col transpose chunks per F
GRP = 4             # super-tiles per output-softplus group (32-aligned PSUM slots)

# (name, K, M) per partition group; lhsT packed block-diagonal [2K, 2M].
W_SPECS = [
    ("W_xl1", D, H), ("W_xin", D, H), ("W_clinm", D, D), ("W_clin", D, H),
    ("W_xl2", H, H), ("W_cp1m", H, H), ("W_cl1m", H, D), ("W_xp1", H, H),
    ("W_cp1", H, H), ("W_cl1", D, H),
    ("W_xl3", H, H), ("W_cp2m", H, H), ("W_cl2m", H, D), ("W_xp2", H, H),
    ("W_cp2", H, H), ("W_cl2", D, H),
    ("W_xlo", H, 1), ("W_cpom", H, H), ("W_clom", H, D),
    ("W_cpo", H, 1), ("W_clo", D, 1),
]
W_KM = {n: (k, m) for n, k, m in W_SPECS}
X0_WEIGHTS = {"W_xl1", "W_xin", "W_clinm"}  # rhs lives at partitions 32:64
W_OFF = {}
_off = 0
for _n, _k, _m in W_SPECS:
    W_OFF[_n] = _off
    _off += 2 * _m
NW = _off

B_SPECS = ["b_xl1", "b_xin", "b_clinm", "b_xl2", "b_cp1m", "b_cl1m", "b_xp1",
           "b_xl3", "b_cp2m", "b_cl2m", "b_xp2", "b_xlo", "b_cpom", "b_clom"]
B_COL = {n: i for i, n in enumerate(B_SPECS)}
NB = len(B_SPECS)


def pack_weights(inputs):
    wpack = np.zeros((128, NW), dtype=ml_dtypes.bfloat16)
    for n, k, m in W_SPECS:
        wt = np.asarray(inputs[n]).astype(np.float32).T  # [K, M]
        assert wt.shape == (k, m), (n, wt.shape)
        wb = wt.astype(ml_dtypes.bfloat16)
        rb = 32 if n in X0_WEIGHTS else 0
        off = W_OFF[n]
        wpack[rb:rb + k, off:off + m] = wb
        wpack[rb + k:rb + 2 * k, off + m:off + 2 * m] = wb
    bpack = np.zeros((128, NB), dtype=np.float32)
    for n in B_SPECS:
        b = np.asarray(inputs[n]).astype(np.float32)
        c = B_COL[n]
        if n == "b_xlo":
            for q in range(GRP):
                bpack[32 * q:32 * q + 2, c] = b[0]
        else:
            L = len(b)
            bpack[0:L, c] = b
            bpack[L:2 * L, c] = b
    ident = np.eye(128, dtype=np.float32)
    return wpack, bpack, ident


def build_program(n_rows):
    assert n_rows % (STB * GRP) == 0
    nst = n_rows // STB
    nc = bacc.Bacc("TRN2", target_bir_lowering=False, debug=False,
                   enable_asserts=False)
    inp_d = nc.dram_tensor("input", [n_rows, 32], dt.float32, kind="ExternalInput")
    w_d = nc.dram_tensor("wpack", [128, NW], dt.bfloat16, kind="ExternalInput")
    b_d = nc.dram_tensor("bpack", [128, NB], dt.float32, kind="ExternalInput")
    c_d = nc.dram_tensor("ident", [128, 128], dt.float32, kind="ExternalInput")
    out_d = nc.dram_tensor("out", [n_rows, 1], dt.float32, kind="ExternalOutput")

    with tile.TileContext(nc) as tc:
        with tc.tile_pool(name="const", bufs=1) as cpool, \
             tc.tile_pool(name="inp", bufs=4) as inpool, \
             tc.tile_pool(name="x0p", bufs=4) as x0pool, \
             tc.tile_pool(name="mh", bufs=6) as mhpool, \
             tc.tile_pool(name="g", bufs=4) as gpool, \
             tc.tile_pool(name="stg", bufs=8) as stgpool, \
             tc.tile_pool(name="axs", bufs=8) as xspool, \
             tc.tile_pool(name="ax", bufs=4) as xpool, \
             tc.tile_pool(name="aout", bufs=2) as outpool, \
             tc.tile_pool(name="ps", bufs=3, space="PSUM") as ps, \
             tc.tile_pool(name="po", bufs=1, space="PSUM") as po:

            wt = cpool.tile([128, NW], dt.bfloat16)
            nc.sync.dma_start(out=wt[:], in_=w_d.ap())
            bt = cpool.tile([128, NB], dt.float32)
            nc.sync.dma_start(out=bt[:], in_=b_d.ap())
            ct = cpool.tile([128, 128], dt.float32)
            nc.sync.dma_start(out=ct[:], in_=c_d.ap())

            def mm_c(psum_t, wname, rhs_t, start, stop, c, ro=0):
                k, m = W_KM[wname]
                off = W_OFF[wname]
                rb = 32 if wname in X0_WEIGHTS else 0
                cs = slice(512 * c, 512 * (c + 1))
                nc.tensor.matmul(psum_t[ro:ro + 2 * m, cs],
                                 wt[rb:rb + 2 * k, off:off + 2 * m],
                                 rhs_t[rb:rb + 2 * k, cs], start=start, stop=stop,
                                 tile_position=(rb, ro) if (rb or ro) else None)

            def mm(psum_t, wname, rhs_t, start, stop, ro=0):
                for c in range(NCHUNK):
                    mm_c(psum_t, wname, rhs_t, start, stop, c, ro)

            def softplus(psum_t, rows, bias_name, out_dtype, pool):
                stg = stgpool.tile([rows, F], dt.float32, tag="stg")
                nc.scalar.activation(stg[0:rows, :], psum_t[0:rows, :], AF.Exp,
                                     bias=bt[0:rows, B_COL[bias_name]:B_COL[bias_name] + 1])
                res = pool.tile([rows, F], out_dtype)
                nc.scalar.activation(res[0:rows, :], stg[0:rows, :], AF.Ln, bias=1.0)
                return res

            def body(st):
                r0 = st * STB
                # ---- input load ----
                # in_t f32 [128, NA*64]; block a: cols 64a+{0:16 x0s-PG0,
                # 16:32 x0s-PG1, 32:48 x0-PG0, 48:64 x0-PG1}. One [128,64]
                # transpose per block lands x0s at rows 0:32 and x0 at rows
                # 32:64 of pT; a single bf16 copy yields x0x (x0s=0:32,
                # x0=32:64, the latter aligned for tile_position=(32,0)).
                in_t = inpool.tile([128, NA * 64], dt.float32, tag="int")
                r3 = in_t[:].rearrange("p (a q) -> p a q", q=64)
                for pg in range(2):
                    rb = r0 + pg * F
                    src_x = inp_d.ap()[rb:rb + F, 0:16].rearrange("(a p) f -> p a f", p=128)
                    src_s = inp_d.ap()[rb:rb + F, 16:32].rearrange("(a p) f -> p a f", p=128)
                    nc.sync.dma_start(out=r3[:, :, 32 + 16 * pg:48 + 16 * pg], in_=src_x)
                    nc.sync.dma_start(out=r3[:, :, 16 * pg:16 * pg + 16], in_=src_s)
                pT = ps.tile([64, F], dt.float32, tag="ps")
                for a in range(NA):
                    nc.tensor.transpose(pT[0:64, 128 * a:128 * (a + 1)],
                                        in_t[:, 64 * a:64 * a + 64], ct[:])
                x0x = x0pool.tile([64, F], dt.bfloat16, tag="x0x")
                nc.vector.tensor_copy(x0x[0:64, :], pT[0:64, :])

                def gate_tail(xs_p, cl_w, cp_w, bcl, dh_p, axs, asv, ro=0):
                    h = mhpool.tile([32, F], dt.bfloat16, tag="mh")
                    nc.vector.scalar_tensor_tensor(
                        h[0:32, :], dh_p[0:32, :],
                        bt[0:32, B_COL[bcl]:B_COL[bcl] + 1],
                        x0x[0:32, :], op0=ALU.add, op1=ALU.mult)
                    for cc in range(NCHUNK):
                        mm_c(xs_p, cl_w, h, False, False, cc, ro)
                    g = gpool.tile([100, F], dt.bfloat16, tag="g")
                    nc.vector.tensor_tensor(g[0:100, :], axs[0:100, :], asv[0:100, :], op=ALU.mult)
                    for cc in range(NCHUNK):
                        mm_c(xs_p, cp_w, g, False, cc == NCHUNK - 1, cc, ro)

                # ---- L1 ----
                p_x1 = ps.tile([100, F], dt.float32, tag="ps")
                mm(p_x1, "W_xin", x0x, True, True)
                a_x1 = softplus(p_x1, 100, "b_xin", dt.bfloat16, xpool)
                p_xs1 = ps.tile([100, F], dt.float32, tag="ps")
                p_dm = ps.tile([32, F], dt.float32, tag="ps")
                mm(p_xs1, "W_xl1", x0x, True, False)
                mm(p_dm, "W_clinm", x0x, True, True)
                m1 = mhpool.tile([32, F], dt.bfloat16, tag="mh")
                nc.vector.scalar_tensor_tensor(
                    m1[0:32, :], p_dm[0:32, :],
                    bt[0:32, B_COL["b_clinm"]:B_COL["b_clinm"] + 1],
                    x0x[0:32, :], op0=ALU.add, op1=ALU.mult)
                for cc in range(NCHUNK):
                    mm_c(p_xs1, "W_clin", m1, False, cc == NCHUNK - 1, cc)
                a_xs1 = softplus(p_xs1, 100, "b_xl1", dt.bfloat16, xspool)
                return dict(st=st, x0x=x0x, a_xs1=a_xs1, a_x1=a_x1, gate_tail=gate_tail)

            po_state = {}

            def back(c):
                st, a_xs1, a_x1, gate_tail = c["st"], c["a_xs1"], c["a_x1"], c["gate_tail"]
                q = st % GRP
                # ---- L2 ----
                p_x2 = ps.tile([100, F], dt.float32, tag="ps")
                mm(p_x2, "W_xp1", a_x1, True, True)
                a_x2 = softplus(p_x2, 100, "b_xp1", dt.bfloat16, xpool)
                p_s1 = ps.tile([100, F], dt.float32, tag="ps")
                mm(p_s1, "W_cp1m", a_x1, True, True)
                a_s1 = softplus(p_s1, 100, "b_cp1m", dt.bfloat16, xspool)
                p_xs2 = ps.tile([100, F], dt.float32, tag="ps")
                p_dh1 = ps.tile([32, F], dt.float32, tag="ps")
                mm(p_xs2, "W_xl2", a_x1, True, False)
                mm(p_dh1, "W_cl1m", a_x1, True, True)
                gate_tail(p_xs2, "W_cl1", "W_cp1", "b_cl1m", p_dh1, a_xs1, a_s1)
                a_xs2 = softplus(p_xs2, 100, "b_xl2", dt.bfloat16, xspool)

                # ---- L3 ----
                p_x3 = ps.tile([100, F], dt.float32, tag="ps")
                mm(p_x3, "W_xp2", a_x2, True, True)
                a_x3 = softplus(p_x3, 100, "b_xp2", dt.bfloat16, xpool)
                p_s2 = ps.tile([100, F], dt.float32, tag="ps")
                mm(p_s2, "W_cp2m", a_x2, True, True)
                a_s2 = softplus(p_s2, 100, "b_cp2m", dt.bfloat16, xspool)
                p_xs3 = ps.tile([100, F], dt.float32, tag="ps")
                p_dh2 = ps.tile([32, F], dt.float32, tag="ps")
                mm(p_xs3, "W_xl3", a_x2, True, False)
                mm(p_dh2, "W_cl2m", a_x2, True, True)
                gate_tail(p_xs3, "W_cl2", "W_cp2", "b_cl2m", p_dh2, a_xs2, a_s2)
                a_xs3 = softplus(p_xs3, 100, "b_xl3", dt.bfloat16, xspool)

                # ---- L4 / output (accumulated into group tile rows 2q:2q+2) ----
                p_s3 = ps.tile([100, F], dt.float32, tag="ps")
                mm(p_s3, "W_cpom", a_x3, True, True)
                a_s3 = softplus(p_s3, 100, "b_cpom", dt.bfloat16, xspool)
                p_dh3 = ps.tile([32, F], dt.float32, tag="ps")
                mm(p_dh3, "W_clom", a_x3, True, True)
                if q == 0:
                    p_grp = po.tile([32 * (GRP - 1) + 2, F], dt.float32, tag="po")
                    po_state["t"] = p_grp
                p_out = po_state["t"]
                mm(p_out, "W_xlo", a_x3, True, False, ro=32 * q)
                gate_tail(p_out, "W_clo", "W_cpo", "b_clom", p_dh3, a_xs3, a_s3, ro=32 * q)

            def group_tail(grp):
                p_out = po_state["t"]
                a_out = softplus(p_out, 32 * (GRP - 1) + 2, "b_xlo", dt.float32, outpool)
                for q in range(GRP):
                    r0 = (grp * GRP + q) * STB
                    nc.sync.dma_start(out=out_d.ap()[r0:r0 + F, 0:1],
                                      in_=a_out[32 * q:32 * q + 1, :])
                    nc.sync.dma_start(out=out_d.ap()[r0 + F:r0 + STB, 0:1],
                                      in_=a_out[32 * q + 1:32 * q + 2, :])

            pending = None
            for st in range(nst):
                c = body(st)
                if pending is not None:
                    back(pending)
                    if pending["st"] % GRP == GRP - 1:
                        group_tail(pending["st"] // GRP)
                pending = c
            back(pending)
            group_tail(pending["st"] // GRP)

    nc.finalize()
    return nc


# ---------------------------------------------------------------------------
# Harness entry point: kernel(**inputs) takes the FULL (unsharded) inputs and
# returns the FULL [B, 1] float32 output. Internally shards the batch across
# the 8 NeuronCores (pure data parallel; weights replicated).
# ---------------------------------------------------------------------------
N_CORES = 8
_program_cache = {}


def _get_program(core_rows):
    if core_rows not in _program_cache:
        _program_cache[core_rows] = build_program(core_rows)
    return _program_cache[core_rows]


def kernel(**inputs):
    from concourse.bass_utils import run_bass_kernel_spmd
    x = np.ascontiguousarray(np.asarray(inputs["input"], dtype=np.float32))
    B = x.shape[0]
    assert x.shape[1] == 2 * D
    core_rows = B // N_CORES
    assert core_rows * N_CORES == B and core_rows % (STB * GRP) == 0, (B,)
    wpack, bpack, ident = pack_weights(inputs)
    nc = _get_program(core_rows)
    in_maps = [{
        "input": x[i * core_rows:(i + 1) * core_rows],
        "wpack": wpack, "bpack": bpack, "ident": ident,
    } for i in range(N_CORES)]
    res = run_bass_kernel_spmd(nc, in_maps, list(range(N_CORES)))
    return np.concatenate([res.results[i]["out"] for i in range(N_CORES)], axis=0)
